# revision 1
# baseline (speedup 1.0000x reference)
"""BiLSTM Trainium2 kernel (8 NeuronCores, SPMD).

Structure (identical program on all cores; per-core data selects the role):
  core 0: fwd-direction layer-0 LSTM   core 2: fwd layer-1 LSTM
  core 1: bwd-direction layer-0 LSTM   core 3: bwd layer-1 LSTM
  cores 4-7: spare (run on zero inputs, results ignored)

Two phases, each = [gather x^T] -> [bulk x@Wx matmul -> gx] -> [recurrence].
Phase 1 gathers token embeddings from the (host-precast bf16) table and runs
layer 0; h0 is written token-major to DRAM and AllGather'd to the partner
core; phase 2 gathers h0 as its "embedding table" and runs layer 1.

Layouts: all matmuls are weight-stationary producing transposed gates
gates^T as PSUM [128p, (m,b)] so the elementwise tail runs at full
128-partition width and h^T feeds the next step's matmul directly.
"""

import numpy as np
import ml_dtypes

B = 16
H = 512
D = 1024
V = 32000
GATE = 4 * H
CH = 16        # recurrence steps per chunk (== one gx block)
TOKB = CH * B  # tokens per gx block = 256

_PROGRAM_CACHE = {}


def build_program(T, stop_after="full"):
    """stop_after: one of 'xmm1', 'rec1', 'coll', 'xmm2', 'full'."""
    import concourse.mybir as mybir
    import concourse.tile as tile
    from concourse import bacc
    from concourse.bass import ds
    from concourse.masks import make_identity
    from concourse.tile_rust import add_dep_helper

    NT = T * B
    NB = T // CH
    GB = min(512, NT)   # tokens per gather block
    NGB = NT // GB
    f32 = mybir.dt.float32
    bf16 = mybir.dt.bfloat16
    i16 = mybir.dt.int16
    Sig = mybir.ActivationFunctionType.Sigmoid
    Tnh = mybir.ActivationFunctionType.Tanh

    nc = bacc.Bacc("TRN2", target_bir_lowering=False, debug=True, num_devices=8)

    tbl = nc.declare_dram_parameter("tbl", [V, D], bf16, isOutput=False)
    ids1 = nc.declare_dram_parameter("ids1", [128, T], i16, isOutput=False)
    ids2 = nc.declare_dram_parameter("ids2", [128, T], i16, isOutput=False)
    wx1 = nc.declare_dram_parameter("wx1", [D, GATE], bf16, isOutput=False)
    wh1 = nc.declare_dram_parameter("wh1", [H, GATE], bf16, isOutput=False)
    bt1 = nc.declare_dram_parameter("bt1", [128, 16], f32, isOutput=False)
    wx2 = nc.declare_dram_parameter("wx2", [H, GATE], bf16, isOutput=False)
    wh2 = nc.declare_dram_parameter("wh2", [H, GATE], bf16, isOutput=False)
    bt2 = nc.declare_dram_parameter("bt2", [128, 16], f32, isOutput=False)
    hT_out = nc.declare_dram_parameter("hT_out", [128, T, 64], f32, isOutput=True)
    h0r = nc.dram_tensor("h0r", [2, NT, H], bf16)
    if stop_after == "xmm1" or stop_after == "xmm2":
        dbg_gx = nc.declare_dram_parameter(
            "dbg_gx", [128, NB, 16, TOKB], f32, isOutput=True
        )
    if stop_after == "rec1":
        dbg_h0 = nc.declare_dram_parameter("dbg_h0", [NT, H], bf16, isOutput=True)
    if stop_after == "coll":
        dbg_h0r = nc.declare_dram_parameter("dbg_h0r", [2, NT, H], bf16, isOutput=True)

    with tile.TileContext(nc) as tc:
        with (
            tc.tile_pool(name="dram", bufs=1, space="DRAM") as dpool,
            tc.tile_pool(name="consts", bufs=1) as cpool,
            tc.tile_pool(name="xin", bufs=2) as xpool,
            tc.tile_pool(name="gxs", bufs=2) as gxpool,
            tc.tile_pool(name="state", bufs=1) as spool,
            tc.tile_pool(name="tmp", bufs=3) as tpool,
            tc.tile_pool(name="hout", bufs=2) as hpool,
            tc.tile_pool(name="ps", bufs=2, space="PSUM") as pspool,
            tc.tile_pool(name="psh", bufs=2, space="PSUM") as pshpool,
        ):
            gx = dpool.tile([128, NB, 16, TOKB], f32)
            h0c = dpool.tile([NT, H], bf16)
            ident = cpool.tile([128, 128], bf16)
            make_identity(nc, ident)

            def emit_xmm(table_ap, elem, nK, ids_in, wx_in, bt_in, dep_inst=None):
                wx_sb = cpool.tile([128, nK, GATE], bf16, tag=f"wx{nK}")
                nc.sync.dma_start(
                    out=wx_sb, in_=wx_in[:, :].rearrange("(k p) m -> p k m", p=128)
                )
                bt_sb = cpool.tile([128, 16], f32, tag="bt")
                nc.sync.dma_start(out=bt_sb, in_=bt_in[:, :])
                ids_sb = cpool.tile([128, T], i16, tag="ids")
                nc.sync.dma_start(out=ids_sb, in_=ids_in[:, :])
                with tc.For_i(0, NGB, 1) as gb:
                    xt = xpool.tile([128, nK, GB], bf16, tag="xt")
                    g_inst = nc.gpsimd.dma_gather(
                        xt[:, :, :],
                        table_ap,
                        ids_sb[:, ds(gb * (GB // 16), GB // 16)],
                        GB,
                        GB,
                        elem,
                        transpose=True,
                    )
                    if dep_inst is not None:
                        add_dep_helper(
                            g_inst.ins, dep_inst.ins, reason="gather after allgather"
                        )
                    for tb in range(GB // TOKB):
                        gxsb = gxpool.tile([128, 1, 16, TOKB], f32, tag="gxsb")
                        for m in range(16):
                            ps = pspool.tile([128, TOKB], f32, tag="psx")
                            for k in range(nK):
                                nc.tensor.matmul(
                                    ps[:, :],
                                    lhsT=wx_sb[:, k, m * 128 : (m + 1) * 128],
                                    rhs=xt[:, k, tb * TOKB : (tb + 1) * TOKB],
                                    start=(k == 0),
                                    stop=(k == nK - 1),
                                )
                            nc.vector.tensor_scalar_add(
                                gxsb[:, 0, m, :], ps[:, :], bt_sb[:, m : m + 1]
                            )
                        nc.sync.dma_start(
                            out=gx[:, ds(gb * (GB // TOKB) + tb, 1), :, :], in_=gxsb
                        )

            def emit_recurrence(wh_in, write_h0c, write_hT):
                wh_sb = cpool.tile([128, 4, GATE], bf16, tag="wh")
                nc.sync.dma_start(
                    out=wh_sb, in_=wh_in[:, :].rearrange("(k p) m -> p k m", p=128)
                )
                c_sb = spool.tile([128, 64], f32, tag="c")
                hT_sb = spool.tile([128, 64], bf16, tag="h")
                nc.vector.memset(c_sb, 0.0)
                nc.vector.memset(hT_sb, 0.0)
                with tc.For_i(0, NB, 1) as bi:
                    gxc = gxpool.tile([128, 1, 16, TOKB], f32, tag="gxc")
                    nc.sync.dma_start(out=gxc, in_=gx[:, ds(bi, 1), :, :])
                    h0st = None
                    hTf = None
                    if write_h0c:
                        h0st = hpool.tile([16, CH, H], bf16, tag="h0st")
                    if write_hT:
                        hTf = hpool.tile([128, CH, 64], f32, tag="hTf")
                    for s in range(CH):
                        psg = pspool.tile([128, 256], f32, tag="psg")
                        for m in range(16):
                            for k in range(4):
                                nc.tensor.matmul(
                                    psg[:, m * 16 : (m + 1) * 16],
                                    lhsT=wh_sb[:, k, m * 128 : (m + 1) * 128],
                                    rhs=hT_sb[:, k * 16 : (k + 1) * 16],
                                    start=(k == 0),
                                    stop=(k == 3),
                                )
                        psg3 = psg.rearrange("p (m b) -> p m b", b=16)
                        nc.vector.tensor_add(
                            psg3, psg3, gxc[:, 0, :, s * 16 : (s + 1) * 16]
                        )
                        i_s = tpool.tile([128, 64], f32, tag="i")
                        g_t = tpool.tile([128, 64], f32, tag="g")
                        f_s = tpool.tile([128, 64], f32, tag="f")
                        o_s = tpool.tile([128, 64], f32, tag="o")
                        nc.scalar.activation(f_s, psg[:, 128:192], Sig)
                        nc.scalar.activation(i_s, psg[:, 0:64], Sig)
                        nc.scalar.activation(g_t, psg[:, 64:128], Tnh)
                        nc.scalar.activation(o_s, psg[:, 192:256], Sig)
                        nc.vector.tensor_mul(c_sb, c_sb, f_s)
                        ig = tpool.tile([128, 64], f32, tag="ig")
                        nc.vector.tensor_mul(ig, i_s, g_t)
                        nc.vector.tensor_add(c_sb, c_sb, ig)
                        th = tpool.tile([128, 64], f32, tag="th")
                        nc.scalar.activation(th, c_sb, Tnh)
                        if write_hT:
                            hf = hTf[:, s, :]
                        else:
                            hf = tpool.tile([128, 64], f32, tag="hf")
                        nc.vector.tensor_mul(hf, o_s, th)
                        nc.vector.tensor_copy(hT_sb, hf)
                        if write_h0c:
                            psh = pshpool.tile([16, H], bf16, tag="psh")
                            for k in range(4):
                                nc.tensor.transpose(
                                    psh[:, k * 128 : (k + 1) * 128],
                                    hT_sb[:, k * 16 : (k + 1) * 16],
                                    ident,
                                )
                            nc.vector.tensor_copy(h0st[:, s, :], psh[:, :])
                    if write_h0c:
                        nc.sync.dma_start(
                            out=h0c[:, :].rearrange("(s b) d -> b s d", b=16)[
                                :, ds(bi * CH, CH), :
                            ],
                            in_=h0st,
                        )
                    if write_hT:
                        nc.sync.dma_start(
                            out=hT_out[:, ds(bi * CH, CH), :], in_=hTf
                        )

            emit_xmm(tbl[:, :], D, 8, ids1, wx1, bt1)
            if stop_after == "xmm1":
                nc.sync.dma_start(out=dbg_gx[:, :, :, :], in_=gx[:, :, :, :])
            else:
                emit_recurrence(wh1, write_h0c=True, write_hT=False)
                if stop_after == "rec1":
                    nc.sync.dma_start(out=dbg_h0[:, :], in_=h0c[:, :])
                else:
                    coll = nc.gpsimd.collective_compute(
                        "AllGather",
                        mybir.AluOpType.bypass,
                        replica_groups=[[0, 2], [1, 3], [4, 6], [5, 7]],
                        ins=[h0c[:, :]],
                        outs=[h0r[:]],
                    )
                    if stop_after == "coll":
                        d2 = nc.sync.dma_start(out=dbg_h0r[:], in_=h0r[:])
                        add_dep_helper(d2.ins, coll.ins, reason="dbg after cc")
                    else:
                        emit_xmm(h0r[0], H, 4, ids2, wx2, bt2, dep_inst=coll)
                        if stop_after == "xmm2":
                            nc.sync.dma_start(
                                out=dbg_gx[:, :, :, :], in_=gx[:, :, :, :]
                            )
                        else:
                            emit_recurrence(wh2, write_h0c=False, write_hT=True)

    nc.finalize()
    return nc


def get_program(T):
    if T not in _PROGRAM_CACHE:
        _PROGRAM_CACHE[T] = build_program(T)
    return _PROGRAM_CACHE[T]


def _bias_T(b):
    """[4H] bias -> [128, 16] transposed-gate layout, haiku f+1 folded in."""
    bv = np.asarray(b, np.float32).copy()
    bv[2 * H : 3 * H] += 1.0
    return np.ascontiguousarray(bv.reshape(16, 128).T)


def _wrap_ids(ids2d):
    """[16, T] int ids -> [128, T] int16 wrap layout (8x replicated)."""
    return np.tile(np.asarray(ids2d).astype(np.int16), (8, 1))


def make_in_maps(input_ids, embed_table, fwd_W0, fwd_b0, fwd_W1, fwd_b1,
                 bwd_W0, bwd_b0, bwd_W1, bwd_b1):
    T = input_ids.shape[1]
    bf = ml_dtypes.bfloat16
    tbl = np.ascontiguousarray(np.asarray(embed_table, np.float32)).astype(bf)

    ids_f = _wrap_ids(input_ids)
    ids_b = _wrap_ids(np.asarray(input_ids)[:, ::-1])
    ident_ids = _wrap_ids(
        (np.arange(T)[None, :] * 16 + np.arange(16)[:, None]).astype(np.int32)
    )

    def wsplit(W, din):
        W = np.asarray(W, np.float32)
        return (
            np.ascontiguousarray(W[:din]).astype(bf),
            np.ascontiguousarray(W[din : din + H]).astype(bf),
        )

    fwx0, fwh0 = wsplit(fwd_W0, D)
    bwx0, bwh0 = wsplit(bwd_W0, D)
    fwx1, fwh1 = wsplit(fwd_W1, H)
    bwx1, bwh1 = wsplit(bwd_W1, H)

    z = np.zeros
    zi16 = z((128, T), np.int16)
    base = dict(
        tbl=z((V, D), bf),
        ids1=zi16, ids2=ident_ids,
        wx1=z((D, GATE), bf), wh1=z((H, GATE), bf), bt1=z((128, 16), np.float32),
        wx2=z((H, GATE), bf), wh2=z((H, GATE), bf), bt2=z((128, 16), np.float32),
    )
    maps = [dict(base) for _ in range(8)]
    # L0 cores
    maps[0].update(tbl=tbl, ids1=ids_f, wx1=fwx0, wh1=fwh0, bt1=_bias_T(fwd_b0))
    maps[1].update(tbl=tbl, ids1=ids_b, wx1=bwx0, wh1=bwh0, bt1=_bias_T(bwd_b0))
    # L1 cores: wx2 is [H, GATE] (no padding needed; phase-2 K loop is 4 chunks)
    maps[2].update(wx2=fwx1, wh2=fwh1, bt2=_bias_T(fwd_b1))
    maps[3].update(wx2=bwx1, wh2=bwh1, bt2=_bias_T(bwd_b1))
    return maps


def assemble_output(hT_fwd, hT_bwd, T):
    def unT(a):
        return (
            np.asarray(a, np.float32)
            .reshape(128, T, 4, 16)
            .transpose(3, 1, 2, 0)
            .reshape(16, T, 512)
        )

    F = unT(hT_fwd)
    Bo = unT(hT_bwd)[:, ::-1, :]
    return np.ascontiguousarray(np.concatenate([F, Bo], axis=2))


def kernel(**inputs):
    from concourse.bass_utils import run_bass_kernel_spmd

    input_ids = np.asarray(inputs["input_ids"])
    T = input_ids.shape[1]
    nc = get_program(T)
    maps = make_in_maps(**inputs)
    res = run_bass_kernel_spmd(nc, maps, list(range(8)))
    out = assemble_output(res.results[2]["hT_out"], res.results[3]["hT_out"], T)
    return out



# revision 8
# speedup vs baseline: 1.7841x; 1.7841x over previous
"""BiLSTM Trainium2 kernel v2 (8 NeuronCores, SPMD, pipelined layers).

Roles (selected at runtime from partition id, same program on all cores):
  core 0: fwd layer-0    core 2: fwd layer-1
  core 1: bwd layer-0    core 3: bwd layer-1
  cores 4-7: spare (zero inputs, outputs ignored)

Single loop nest over NL superblock-steps; every loop body holds
[recurrence block of superblock t-1] + [x@Wx matmul block of superblock t]
so the bulk matmuls fill TensorE gaps left by the sequential recurrence.
Layer-0 output h0 is written in transposed (hT) layout and AllGather'd to
the partner layer-1 core once per superblock; the layer-1 core consumes it
LAG superblocks later (uniform collective placement keeps all cores issuing
identical collectives in identical order). The only role-divergent code is
the x-tile source (embedding gather vs h0 DMA), one state reset, and a
pid-derived output offset.

Gate order is permuted host-side to [i, f, o, g] so one fused sigmoid
covers i|f|o and one tanh covers g. h stays transposed everywhere: the
recurrence's weight-stationary matmuls produce gates^T in PSUM at full
128-partition width, h^T feeds the next step directly, and layer-0's h^T
chunks are DMA'd straight to DRAM (no PE transposes anywhere).
"""

import numpy as np
import ml_dtypes

B = 16
H = 512
D = 1024
V = 32000
GATE = 4 * H
CH = 16            # recurrence steps per block
TOKB = CH * B      # tokens per block = 256
SB = 4             # blocks per superblock
SBTOK = SB * TOKB  # tokens per superblock = 1024
LAG = 3            # consumer lag in superblocks
PADB = 256         # hT_out block slots (pow2 so % is cheap); > NB + LAG*SB

_PROGRAM_CACHE = {}


def build_program(T):
    import concourse.mybir as mybir
    import concourse.tile as tile
    from concourse import bacc
    from concourse.bass import ds
    from concourse.tile_rust import add_dep_helper

    NT = T * B
    NB = NT // TOKB
    NSB = NB // SB
    NL = NSB + 1 + LAG

    f32 = mybir.dt.float32
    bf16 = mybir.dt.bfloat16
    i16 = mybir.dt.int16
    Sig = mybir.ActivationFunctionType.Sigmoid
    Tnh = mybir.ActivationFunctionType.Tanh

    nc = bacc.Bacc("TRN2", target_bir_lowering=False, debug=True, num_devices=8)

    tbl = nc.declare_dram_parameter("tbl", [V, D], bf16, isOutput=False)
    ids = nc.declare_dram_parameter("ids", [128, T], i16, isOutput=False)
    wx = nc.declare_dram_parameter("wx", [D, GATE], bf16, isOutput=False)
    wh = nc.declare_dram_parameter("wh", [H, GATE], bf16, isOutput=False)
    bt = nc.declare_dram_parameter("bt", [128, 16], f32, isOutput=False)
    hT_out = nc.declare_dram_parameter(
        "hT_out", [128, 4, PADB * TOKB], bf16, isOutput=True
    )

    with tile.TileContext(nc) as tc:
        with (
            tc.tile_pool(name="dram", bufs=1, space="DRAM") as dpool,
            tc.tile_pool(name="consts", bufs=1) as cpool,
            tc.tile_pool(name="xin", bufs=2) as xpool,
            tc.tile_pool(name="gxf", bufs=2) as gxpool,
            tc.tile_pool(name="gxc", bufs=2) as gcpool,
            tc.tile_pool(name="state", bufs=1) as spool,
            tc.tile_pool(name="tmp", bufs=3) as tpool,
            tc.tile_pool(name="hout", bufs=2) as hpool,
            tc.tile_pool(name="ps", bufs=2, space="PSUM") as pspool,
            tc.tile_pool(name="psx", bufs=2, space="PSUM") as psxpool,
        ):
            h0x = [dpool.tile([128, 4, SBTOK], bf16, tag=f"h0x{j}", name=f"h0x{j}") for j in range(NSB)]
            h0r = [
                dpool.tile([2, 128, 4, SBTOK], bf16, tag=f"h0r{j}", name=f"h0r{j}")
                for j in range(NSB)
            ]
            gxb = [
                dpool.tile([128, SB, 16, TOKB], bf16, tag=f"gxb{p}", name=f"gxb{p}")
                for p in range(2)
            ]

            pid = nc.partition_id()
            # 0 on layer-0 cores, LAG*SB on layer-1 cores (hT_out block offset)
            ofs_sv = nc.snap(((pid // 2) % 2) * (LAG * SB))

            wx_sb = cpool.tile([128, 8, GATE], bf16, tag="wx")
            nc.sync.dma_start(
                out=wx_sb, in_=wx[:, :].rearrange("(k p) m -> p k m", p=128)
            )
            wh_sb = cpool.tile([128, 4, GATE], bf16, tag="wh")
            nc.sync.dma_start(
                out=wh_sb, in_=wh[:, :].rearrange("(k p) m -> p k m", p=128)
            )
            bt_sb = cpool.tile([128, 16], f32, tag="bt")
            nc.sync.dma_start(out=bt_sb, in_=bt[:, :])
            ids_sb = cpool.tile([128, T], i16, tag="ids")
            nc.sync.dma_start(out=ids_sb, in_=ids[:, :])

            c_sb = spool.tile([128, 64], f32, tag="c")
            hT_sb = spool.tile([128, 4, 16], bf16, tag="h")
            nc.vector.memset(c_sb, 0.0)
            nc.vector.memset(hT_sb, 0.0)

            # zero the consumed half of the h0r buffers read during pipeline
            # warmup (before any AllGather has filled them)
            zt = cpool.tile([128, 4, SBTOK], bf16, tag="zt")
            nc.vector.memset(zt, 0.0)
            for j in range(min(LAG, NSB)):
                src = (j - LAG) % NSB
                nc.sync.dma_start(out=h0r[src][0], in_=zt)

            colls = {}
            xts = {}

            def emit_load(tt):
                """Stage the x-input tile for loop tt (issued two loops early)."""
                xt = xpool.tile([128, 2, 8, 512], bf16, tag="xt")
                xts[tt] = xt
                with tc.If((pid % 4) < 2) as cmp:
                    for g in range(2):
                        nc.gpsimd.dma_gather(
                            xt[:, g, :, :],
                            tbl[:, :],
                            ids_sb[:, ds(((tt % NSB) * SB) * CH + g * 32, 32)],
                            512,
                            512,
                            D,
                            transpose=True,
                        )
                with cmp.Else():
                    nc.vector.memset(xt[:, :, 4:8, :], 0.0)
                    src = (tt - LAG) % NSB
                    d = None
                    for g in range(2):
                        d = nc.sync.dma_start(
                            out=xt[:, g, 0:4, :],
                            in_=h0r[src][0][:, :, g * 512 : (g + 1) * 512],
                        )
                        if 0 <= tt - LAG < NSB and (tt - LAG) in colls:
                            add_dep_helper(
                                d.ins, colls[tt - LAG].ins, reason="xt after allgather"
                            )

            def emit_rec_body(t, i):
                """Recurrence for block i of superblock t-1."""
                sbi = (t - 1) % NSB
                gxc = gcpool.tile([128, 16, TOKB], bf16, tag="gxc")
                nc.sync.dma_start(out=gxc, in_=gxb[(t - 1) % 2][:, ds(i, 1), :, :])
                hTf = hpool.tile([128, 4, CH, 16], bf16, tag="hTf")
                for s in range(CH):
                    psg = pspool.tile([128, 256], f32, tag="psg")
                    for m in range(16):
                        for k in range(4):
                            rhs = hT_sb[:, k, :] if s == 0 else hTf[:, k, s - 1, :]
                            nc.tensor.matmul(
                                psg[:, m * 16 : (m + 1) * 16],
                                lhsT=wh_sb[:, k, m * 128 : (m + 1) * 128],
                                rhs=rhs,
                                start=(k == 0),
                                stop=(k == 3),
                            )
                    psg3 = psg.rearrange("p (m b) -> p m b", b=16)
                    nc.vector.tensor_add(
                        psg3[:, 0:12, :],
                        psg3[:, 0:12, :],
                        gxc[:, 0:12, s * 16 : (s + 1) * 16],
                    )
                    nc.vector.tensor_add(
                        psg3[:, 12:16, :],
                        psg3[:, 12:16, :],
                        gxc[:, 12:16, s * 16 : (s + 1) * 16],
                    )
                    sg = tpool.tile([128, 192], f32, tag="sg")
                    nc.scalar.activation(sg, psg[:, 0:192], Sig)
                    tg = tpool.tile([128, 64], f32, tag="tg")
                    nc.scalar.activation(tg, psg[:, 192:256], Tnh)
                    cf = tpool.tile([128, 64], f32, tag="cf")
                    nc.vector.tensor_mul(cf, c_sb, sg[:, 64:128])
                    ig = tpool.tile([128, 64], f32, tag="ig")
                    nc.vector.tensor_mul(ig, sg[:, 0:64], tg)
                    nc.vector.tensor_add(c_sb, cf, ig)
                    th = tpool.tile([128, 64], f32, tag="th")
                    nc.scalar.activation(th, c_sb, Tnh)
                    nc.vector.tensor_mul(hTf[:, :, s, :], sg[:, 128:192], th)
                    if s == CH - 1:
                        nc.vector.tensor_copy(hT_sb, hTf[:, :, s, :])
                nc.sync.dma_start(
                    out=h0x[sbi][:, :, ds(i * TOKB, TOKB)], in_=hTf
                )
                goff = ((t - 1) * SB + PADB + i - ofs_sv) % PADB
                nc.sync.dma_start(
                    out=hT_out[:, :, ds(goff * TOKB, TOKB)], in_=hTf
                )

            def emit_xmm_body(t, i):
                """x @ Wx for block i of superblock t -> gxb[t % 2]."""
                xt = xts[t]
                gxf = gxpool.tile([128, 16, TOKB], bf16, tag="gxf")
                for m in range(16):
                    ps = psxpool.tile([128, TOKB], f32, tag="psx")
                    for k in range(8):
                        nc.tensor.matmul(
                            ps[:, :],
                            lhsT=wx_sb[:, k, m * 128 : (m + 1) * 128],
                            rhs=xt[:, ds(i // 2, 1), k, ds((i % 2) * TOKB, TOKB)],
                            start=(k == 0),
                            stop=(k == 7),
                        )
                    nc.vector.tensor_scalar_add(
                        gxf[:, m, :], ps[:, :], bt_sb[:, m : m + 1]
                    )
                nc.sync.dma_start(out=gxb[t % 2][:, ds(i, 1), :, :], in_=gxf)

            emit_load(0)
            emit_load(1)
            for t in range(NL):
                with tc.For_i(0, SB, 1) as i:
                    if t > 0:
                        emit_rec_body(t, i)
                    if t < NL - 1:
                        emit_xmm_body(t, i)
                j = t - 1
                if 0 <= j < NSB:
                    colls[j] = nc.gpsimd.collective_compute(
                        "AllGather",
                        mybir.AluOpType.bypass,
                        replica_groups=[[0, 2], [1, 3], [4, 6], [5, 7]],
                        ins=[h0x[j][:]],
                        outs=[h0r[j][:]],
                    )
                if t + 2 < NL:
                    emit_load(t + 2)
                if t == LAG:
                    # layer-1 cores start their real recurrence next loop
                    with tc.If((pid % 4) >= 2):
                        nc.vector.memset(c_sb, 0.0)
                        nc.vector.memset(hT_sb, 0.0)

    nc.finalize()
    return nc


def get_program(T):
    if T not in _PROGRAM_CACHE:
        _PROGRAM_CACHE[T] = build_program(T)
    return _PROGRAM_CACHE[T]


# gate reorder: reference layout [i, g, f, o] -> kernel layout [i, f, o, g]
_PERM = np.r_[0:512, 1024:1536, 1536:2048, 512:1024]


def _prep_weights(W, b, din):
    """W [din+H, 4H], b [4H] -> (wx [D,GATE] bf16, wh [H,GATE] bf16, bt [128,16] f32)."""
    bf = ml_dtypes.bfloat16
    W = np.asarray(W, np.float32)[:, _PERM]
    bv = np.asarray(b, np.float32)[_PERM].copy()
    bv[512:1024] += 1.0  # haiku forget-gate +1 (f block now at 512:1024)
    wxp = np.zeros((D, GATE), np.float32)
    wxp[0:din] = W[0:din]
    whp = np.ascontiguousarray(W[din : din + H])
    btp = np.ascontiguousarray(bv.reshape(16, 128).T)
    return wxp.astype(bf), whp.astype(bf), btp


def _wrap_ids(ids2d):
    return np.tile(np.asarray(ids2d).astype(np.int16), (8, 1))


def make_in_maps(input_ids, embed_table, fwd_W0, fwd_b0, fwd_W1, fwd_b1,
                 bwd_W0, bwd_b0, bwd_W1, bwd_b1):
    T = input_ids.shape[1]
    bf = ml_dtypes.bfloat16
    tbl = np.ascontiguousarray(np.asarray(embed_table, np.float32)).astype(bf)

    ids_f = _wrap_ids(input_ids)
    ids_b = _wrap_ids(np.asarray(input_ids)[:, ::-1])

    z = np.zeros
    base = dict(
        tbl=z((V, D), bf),
        ids=z((128, T), np.int16),
        wx=z((D, GATE), bf),
        wh=z((H, GATE), bf),
        bt=z((128, 16), np.float32),
    )
    maps = [dict(base) for _ in range(8)]

    fx0, fh0, fb0t = _prep_weights(fwd_W0, fwd_b0, D)
    bx0, bh0, bb0t = _prep_weights(bwd_W0, bwd_b0, D)
    fx1, fh1, fb1t = _prep_weights(fwd_W1, fwd_b1, H)
    bx1, bh1, bb1t = _prep_weights(bwd_W1, bwd_b1, H)

    maps[0].update(tbl=tbl, ids=ids_f, wx=fx0, wh=fh0, bt=fb0t)
    maps[1].update(tbl=tbl, ids=ids_b, wx=bx0, wh=bh0, bt=bb0t)
    maps[2].update(wx=fx1, wh=fh1, bt=fb1t)
    maps[3].update(wx=bx1, wh=bh1, bt=bb1t)
    return maps


def assemble_output(hT_fwd, hT_bwd, T):
    def unT(a):
        arr = np.asarray(a, np.float32)[:, :, : T * 16].reshape(128, 4, T, 16)
        return np.ascontiguousarray(arr.transpose(3, 2, 1, 0).reshape(16, T, 512))

    F = unT(hT_fwd)
    Bo = unT(hT_bwd)[:, ::-1, :]
    return np.ascontiguousarray(np.concatenate([F, Bo], axis=2))


def kernel(**inputs):
    from concourse.bass_utils import run_bass_kernel_spmd

    input_ids = np.asarray(inputs["input_ids"])
    T = input_ids.shape[1]
    nc = get_program(T)
    maps = make_in_maps(**inputs)
    res = run_bass_kernel_spmd(nc, maps, list(range(8)))
    return assemble_output(
        res.results[2]["hT_out"], res.results[3]["hT_out"], T
    )


# revision 12
# speedup vs baseline: 2.2454x; 1.2585x over previous
"""BiLSTM Trainium2 kernel v2 (8 NeuronCores, SPMD, pipelined layers).

Roles (selected at runtime from partition id, same program on all cores):
  core 0: fwd layer-0    core 2: fwd layer-1
  core 1: bwd layer-0    core 3: bwd layer-1
  cores 4-7: spare (zero inputs, outputs ignored)

Single loop nest over NL superblock-steps; every loop body holds
[recurrence block of superblock t-1] + [x@Wx matmul block of superblock t]
so the bulk matmuls fill TensorE gaps left by the sequential recurrence.
Layer-0 output h0 is written in transposed (hT) layout and AllGather'd to
the partner layer-1 core once per superblock; the layer-1 core consumes it
LAG superblocks later (uniform collective placement keeps all cores issuing
identical collectives in identical order). The only role-divergent code is
the x-tile source (embedding gather vs h0 DMA), one state reset, and a
pid-derived output offset.

Gate order is permuted host-side to [i, f, o, g] so one fused sigmoid
covers i|f|o and one tanh covers g. h stays transposed everywhere: the
recurrence's weight-stationary matmuls produce gates^T in PSUM at full
128-partition width, h^T feeds the next step directly, and layer-0's h^T
chunks are DMA'd straight to DRAM (no PE transposes anywhere).
"""

import numpy as np
import ml_dtypes

B = 16
H = 512
D = 1024
V = 32000
GATE = 4 * H
CH = 16            # recurrence steps per block
TOKB = CH * B      # tokens per block = 256
SB = 4             # blocks per superblock
SBTOK = SB * TOKB  # tokens per superblock = 1024
LAG = 3            # consumer lag in superblocks
PADB = 256         # hT_out block slots (pow2 so % is cheap); > NB + LAG*SB

_PROGRAM_CACHE = {}


def build_program(T):
    import concourse.mybir as mybir
    import concourse.tile as tile
    from concourse import bacc
    from concourse.bass import ds
    from concourse.masks import make_identity
    from concourse.tile_rust import add_dep_helper

    NT = T * B
    NB = NT // TOKB
    NSB = NB // SB
    NL = NSB + 1 + LAG

    f32 = mybir.dt.float32
    bf16 = mybir.dt.bfloat16
    i16 = mybir.dt.int16
    Sig = mybir.ActivationFunctionType.Sigmoid
    Tnh = mybir.ActivationFunctionType.Tanh
    Cpy = mybir.ActivationFunctionType.Identity

    nc = bacc.Bacc("TRN2", target_bir_lowering=False, debug=True, num_devices=8)

    tbl = nc.declare_dram_parameter("tbl", [V, D], bf16, isOutput=False)
    ids = nc.declare_dram_parameter("ids", [128, T], i16, isOutput=False)
    wx = nc.declare_dram_parameter("wx", [D, GATE], bf16, isOutput=False)
    wh = nc.declare_dram_parameter("wh", [H, GATE], bf16, isOutput=False)
    bt = nc.declare_dram_parameter("bt", [128, 16], f32, isOutput=False)
    hT_out = nc.declare_dram_parameter(
        "hT_out", [128, 4, PADB * TOKB], bf16, isOutput=True
    )

    with tile.TileContext(nc) as tc:
        with (
            tc.tile_pool(name="dram", bufs=1, space="DRAM") as dpool,
            tc.tile_pool(name="consts", bufs=1) as cpool,
            tc.tile_pool(name="xin", bufs=2) as xpool,
            tc.tile_pool(name="gxf", bufs=2) as gxpool,
            tc.tile_pool(name="gxc", bufs=2) as gcpool,
            tc.tile_pool(name="state", bufs=1) as spool,
            tc.tile_pool(name="tmp", bufs=3) as tpool,
            tc.tile_pool(name="hout", bufs=2) as hpool,
            tc.tile_pool(name="ps", bufs=2, space="PSUM") as pspool,
            tc.tile_pool(name="ps2", bufs=2, space="PSUM") as ps2pool,
            tc.tile_pool(name="ps3", bufs=2, space="PSUM") as ps3pool,
            tc.tile_pool(name="psx", bufs=2, space="PSUM") as psxpool,
        ):
            h0x = [dpool.tile([128, 4, SBTOK], bf16, tag=f"h0x{j}", name=f"h0x{j}") for j in range(NSB)]
            h0r = [
                dpool.tile([2, 128, 4, SBTOK], bf16, tag=f"h0r{j}", name=f"h0r{j}")
                for j in range(NSB)
            ]
            gxb = [
                dpool.tile([128, SB, CH, 16, B], bf16, tag=f"gxb{p}", name=f"gxb{p}")
                for p in range(2)
            ]

            pid = nc.partition_id()
            # 0 on layer-0 cores, LAG*SB on layer-1 cores (hT_out block offset)
            ofs_sv = nc.snap(((pid // 2) % 2) * (LAG * SB))

            wx_sb = cpool.tile([128, 8, GATE], bf16, tag="wx")
            nc.sync.dma_start(
                out=wx_sb, in_=wx[:, :].rearrange("(k p) m -> p k m", p=128)
            )
            wh_sb = cpool.tile([128, 4, GATE], bf16, tag="wh")
            nc.sync.dma_start(
                out=wh_sb, in_=wh[:, :].rearrange("(k p) m -> p k m", p=128)
            )
            bt_sb = cpool.tile([128, 16], f32, tag="bt")
            nc.sync.dma_start(out=bt_sb, in_=bt[:, :])
            ids_sb = cpool.tile([128, T], i16, tag="ids")
            nc.sync.dma_start(out=ids_sb, in_=ids[:, :])
            ident = cpool.tile([128, 128], bf16, tag="ident")
            make_identity(nc, ident)

            c_sb = spool.tile([128, 64], f32, tag="c")
            hT_sb = spool.tile([128, 4, 16], bf16, tag="h")
            nc.vector.memset(c_sb, 0.0)
            nc.vector.memset(hT_sb, 0.0)

            # zero the consumed half of the h0r buffers read during pipeline
            # warmup (before any AllGather has filled them)
            zt = cpool.tile([128, 4, SBTOK], bf16, tag="zt")
            nc.vector.memset(zt, 0.0)
            for j in range(min(LAG, NSB)):
                src = (j - LAG) % NSB
                nc.sync.dma_start(out=h0r[src][0], in_=zt)

            colls = {}
            xts = {}

            def emit_load(tt):
                """Stage the x-input tile for loop tt (issued two loops early)."""
                xt = xpool.tile([128, 2, 8, 512], bf16, tag="xt")
                xts[tt] = xt
                with tc.If((pid % 4) < 2) as cmp:
                    for g in range(2):
                        nc.gpsimd.dma_gather(
                            xt[:, g, :, :],
                            tbl[:, :],
                            ids_sb[:, ds(((tt % NSB) * SB) * CH + g * 32, 32)],
                            512,
                            512,
                            D,
                            transpose=True,
                        )
                with cmp.Else():
                    nc.vector.memset(xt[:, :, 4:8, :], 0.0)
                    src = (tt - LAG) % NSB
                    d = None
                    for g in range(2):
                        d = nc.sync.dma_start(
                            out=xt[:, g, 0:4, :],
                            in_=h0r[src][0][:, :, g * 512 : (g + 1) * 512],
                        )
                        if 0 <= tt - LAG < NSB and (tt - LAG) in colls:
                            add_dep_helper(
                                d.ins, colls[tt - LAG].ins, reason="xt after allgather"
                            )

            def emit_rec_body(t, i):
                """Recurrence for block i of superblock t-1."""
                sbi = (t - 1) % NSB
                gxc = gcpool.tile([128, CH, 16, B], bf16, tag="gxc")
                nc.sync.dma_start(out=gxc, in_=gxb[(t - 1) % 2][:, ds(i, 1), :, :, :])
                hTf = hpool.tile([128, 4, CH, 16], bf16, tag="hTf")
                for s in range(CH):
                    # three PSUM banks so early gate groups are readable while
                    # the PE is still accumulating later ones
                    psg_g = pspool.tile([128, 64], f32, tag="psg_g")
                    psg_if = ps2pool.tile([128, 128], f32, tag="psg_if")
                    psg_o = ps3pool.tile([128, 64], f32, tag="psg_o")
                    groups = [
                        (psg_g, 0, 4),
                        (psg_if, 4, 12),
                        (psg_o, 12, 16),
                    ]
                    for ptile, m0, m1 in groups:
                        nc.tensor.matmul(
                            ptile[:, :],
                            lhsT=ident,
                            rhs=gxc[:, s, m0:m1, :],
                            start=True,
                            stop=False,
                        )
                        for m in range(m0, m1):
                            for k in range(4):
                                rhs = hT_sb[:, k, :] if s == 0 else hTf[:, k, s - 1, :]
                                nc.tensor.matmul(
                                    ptile[:, (m - m0) * 16 : (m - m0 + 1) * 16],
                                    lhsT=wh_sb[:, k, m * 128 : (m + 1) * 128],
                                    rhs=rhs,
                                    start=False,
                                    stop=(m == m1 - 1 and k == 3),
                                )
                    tg = tpool.tile([128, 64], f32, tag="tg")
                    nc.scalar.activation(tg, psg_g[:, :], Tnh)
                    sif = tpool.tile([128, 128], f32, tag="sif")
                    nc.scalar.activation(sif, psg_if[:, :], Sig)
                    so = tpool.tile([128, 64], f32, tag="so")
                    nc.scalar.activation(so, psg_o[:, :], Sig)
                    ig = tpool.tile([128, 64], f32, tag="ig")
                    nc.vector.tensor_mul(ig, sif[:, 0:64], tg)
                    cf = tpool.tile([128, 64], f32, tag="cf")
                    nc.vector.tensor_mul(cf, c_sb, sif[:, 64:128])
                    nc.vector.tensor_add(c_sb, cf, ig)
                    th = tpool.tile([128, 64], f32, tag="th")
                    nc.scalar.activation(th, c_sb, Tnh)
                    nc.vector.tensor_mul(hTf[:, :, s, :], so, th)
                    if s == CH - 1:
                        nc.vector.tensor_copy(hT_sb, hTf[:, :, s, :])
                nc.sync.dma_start(
                    out=h0x[sbi][:, :, ds(i * TOKB, TOKB)], in_=hTf
                )
                goff = ((t - 1) * SB + PADB + i - ofs_sv) % PADB
                nc.sync.dma_start(
                    out=hT_out[:, :, ds(goff * TOKB, TOKB)], in_=hTf
                )

            def emit_xmm_body(t, i):
                """x @ Wx for block i of superblock t -> gxb[t % 2]."""
                xt = xts[t]
                gxf = gxpool.tile([128, CH, 16, B], bf16, tag="gxf")
                for m in range(16):
                    ps = psxpool.tile([128, TOKB], f32, tag="psx")
                    for k in range(8):
                        nc.tensor.matmul(
                            ps[:, :],
                            lhsT=wx_sb[:, k, m * 128 : (m + 1) * 128],
                            rhs=xt[:, ds(i // 2, 1), k, ds((i % 2) * TOKB, TOKB)],
                            start=(k == 0),
                            stop=(k == 7),
                        )
                    nc.scalar.activation(
                        gxf[:, :, m, :], ps[:, :], Cpy, bias=bt_sb[:, m : m + 1]
                    )
                nc.sync.dma_start(out=gxb[t % 2][:, ds(i, 1), :, :, :], in_=gxf)

            emit_load(0)
            emit_load(1)
            for t in range(NL):
                with tc.For_i(0, SB, 1) as i:
                    if t > 0:
                        emit_rec_body(t, i)
                    if t < NL - 1:
                        emit_xmm_body(t, i)
                j = t - 1
                if 0 <= j < NSB:
                    colls[j] = nc.gpsimd.collective_compute(
                        "AllGather",
                        mybir.AluOpType.bypass,
                        replica_groups=[[0, 2], [1, 3], [4, 6], [5, 7]],
                        ins=[h0x[j][:]],
                        outs=[h0r[j][:]],
                    )
                if t + 2 < NL:
                    emit_load(t + 2)
                if t == LAG:
                    # layer-1 cores start their real recurrence next loop
                    with tc.If((pid % 4) >= 2):
                        nc.vector.memset(c_sb, 0.0)
                        nc.vector.memset(hT_sb, 0.0)

    nc.finalize()
    return nc


def get_program(T):
    if T not in _PROGRAM_CACHE:
        _PROGRAM_CACHE[T] = build_program(T)
    return _PROGRAM_CACHE[T]


# gate reorder: reference layout [i, g, f, o] -> kernel layout [g, i, f, o]
_PERM = np.r_[512:1024, 0:512, 1024:1536, 1536:2048]


def _prep_weights(W, b, din):
    """W [din+H, 4H], b [4H] -> (wx [D,GATE] bf16, wh [H,GATE] bf16, bt [128,16] f32)."""
    bf = ml_dtypes.bfloat16
    W = np.asarray(W, np.float32)[:, _PERM]
    bv = np.asarray(b, np.float32)[_PERM].copy()
    bv[1024:1536] += 1.0  # haiku forget-gate +1 (f block now at 1024:1536)
    wxp = np.zeros((D, GATE), np.float32)
    wxp[0:din] = W[0:din]
    whp = np.ascontiguousarray(W[din : din + H])
    btp = np.ascontiguousarray(bv.reshape(16, 128).T)
    return wxp.astype(bf), whp.astype(bf), btp


def _wrap_ids(ids2d):
    return np.tile(np.asarray(ids2d).astype(np.int16), (8, 1))


def make_in_maps(input_ids, embed_table, fwd_W0, fwd_b0, fwd_W1, fwd_b1,
                 bwd_W0, bwd_b0, bwd_W1, bwd_b1):
    T = input_ids.shape[1]
    bf = ml_dtypes.bfloat16
    tbl = np.ascontiguousarray(np.asarray(embed_table, np.float32)).astype(bf)

    ids_f = _wrap_ids(input_ids)
    ids_b = _wrap_ids(np.asarray(input_ids)[:, ::-1])

    z = np.zeros
    base = dict(
        tbl=z((V, D), bf),
        ids=z((128, T), np.int16),
        wx=z((D, GATE), bf),
        wh=z((H, GATE), bf),
        bt=z((128, 16), np.float32),
    )
    maps = [dict(base) for _ in range(8)]

    fx0, fh0, fb0t = _prep_weights(fwd_W0, fwd_b0, D)
    bx0, bh0, bb0t = _prep_weights(bwd_W0, bwd_b0, D)
    fx1, fh1, fb1t = _prep_weights(fwd_W1, fwd_b1, H)
    bx1, bh1, bb1t = _prep_weights(bwd_W1, bwd_b1, H)

    maps[0].update(tbl=tbl, ids=ids_f, wx=fx0, wh=fh0, bt=fb0t)
    maps[1].update(tbl=tbl, ids=ids_b, wx=bx0, wh=bh0, bt=bb0t)
    maps[2].update(wx=fx1, wh=fh1, bt=fb1t)
    maps[3].update(wx=bx1, wh=bh1, bt=bb1t)
    return maps


def assemble_output(hT_fwd, hT_bwd, T):
    def unT(a):
        arr = np.asarray(a, np.float32)[:, :, : T * 16].reshape(128, 4, T, 16)
        return np.ascontiguousarray(arr.transpose(3, 2, 1, 0).reshape(16, T, 512))

    F = unT(hT_fwd)
    Bo = unT(hT_bwd)[:, ::-1, :]
    return np.ascontiguousarray(np.concatenate([F, Bo], axis=2))


def kernel(**inputs):
    from concourse.bass_utils import run_bass_kernel_spmd

    input_ids = np.asarray(inputs["input_ids"])
    T = input_ids.shape[1]
    nc = get_program(T)
    maps = make_in_maps(**inputs)
    res = run_bass_kernel_spmd(nc, maps, list(range(8)))
    return assemble_output(
        res.results[2]["hT_out"], res.results[3]["hT_out"], T
    )
